# revision 6
# baseline (speedup 1.0000x reference)
"""MeshGraphNet on 8 TRN2 NeuronCores (Bass/Tile).

Strategy: edge-parallel. Edges are sorted by dst on the host and split into 8
contiguous node ranges (so each core's segment-sum is fully local). Node MLPs
run data-parallel on each core's node range; a bf16 AllGather replicates node
latents each iteration for the h[src]/h[dst] gathers (dma_gather transpose
mode gathers straight into feature-major tiles). All activations are
feature-major [128 feat, n] so MLP layers chain without transposes; matmuls
run in float32r (FP22). LayerNorm mean-subtraction is folded into a single
centering matmul Cg = diag(gamma) @ (I - J/128) with host-precomputed
weights. Segment-sum = one-hot is_equal masks x PE matmuls accumulated in
PSUM per 256-node pair.
"""
import sys

sys.path.insert(0, "/opt/trn_rl_repo")
import numpy as np
import ml_dtypes

import concourse.bacc as bacc
import concourse.tile as tile
import concourse.mybir as mybir
from concourse.bass_utils import run_bass_kernel_spmd

F32 = mybir.dt.float32
F32R = mybir.dt.float32r
BF16 = mybir.dt.bfloat16
I16 = mybir.dt.int16
LR = mybir.ActivationFunctionType.Lrelu
IDENT = mybir.ActivationFunctionType.Identity
SQUARE = mybir.ActivationFunctionType.Square
SQRT = mybir.ActivationFunctionType.Sqrt
EQ = mybir.AluOpType.is_equal
ADD = mybir.AluOpType.add
MULT = mybir.AluOpType.mult

NN = 20000
NE = 320000
L = 128
H = 256
ITERS = 3
IN_NODE = 12
IN_EDGE = 4
OUT = 2
EPS = 1e-5
CORES = 8
TE = 512          # edges per tile
NPC = 2560        # padded nodes per core (20 groups of 128, 10 pairs of 256)
NPAIR = NPC // 256
ALPHA = 0.01      # leaky relu slope


# ---------------------------------------------------------------- host prep

class _Packer:
    """Packs weights into a few [128, X] matrices; device slices columns."""

    def __init__(self, dtype):
        self.cols = []
        self.x = 0
        self.dtype = dtype

    def add(self, arr):  # arr [k, w] with k <= 128
        arr = np.asarray(arr, np.float32)
        if arr.ndim == 1:
            arr = arr[:, None]
        k, w = arr.shape
        assert k <= 128
        off = self.x
        self.cols.append((off, arr))
        self.x += w
        return off

    def build(self):
        m = np.zeros((128, self.x), np.float32)
        for off, arr in self.cols:
            m[: arr.shape[0], off: off + arr.shape[1]] = arr
        return m.astype(self.dtype)


def _prep_mlp(wf, wb, wsc, p, in_blocks_bf):
    """Pack one MLP's params. in_blocks_bf: list of bools, True -> pack that
    128-row Wi block in bf16 (for bf16 rhs operands)."""
    wi = np.asarray(p["Wi"], np.float32)
    bi = np.asarray(p["bi"], np.float32)
    wh = np.asarray(p["Wh"][0], np.float32)
    bh = np.asarray(p["bh"][0], np.float32)
    wo = np.asarray(p["Wo"], np.float32)
    bo = np.asarray(p["bo"], np.float32)
    outf = wo.shape[1]
    in_f = wi.shape[0]
    blocks = []
    r = 0
    for bi_ix, is_bf in enumerate(in_blocks_bf):
        kb = min(128, in_f - r)
        blk = wi[r: r + kb, :]
        if is_bf:
            blocks.append(("wb", wb.add(blk), kb))
        else:
            blocks.append(("wf", wf.add(blk), kb))
        r += kb
    assert r == in_f
    m = {
        "blocks": blocks,
        "wh": wf.add(np.concatenate([wh[0:128, :], wh[128:256, :]], axis=1)),
        "wo": wf.add(np.concatenate([wo[0:128, :], wo[128:256, :]], axis=1)),
        "b1": wsc.add(np.stack([bi[0:128], bi[128:256]], axis=1)),
        "b2": wsc.add(np.stack([bh[0:128], bh[128:256]], axis=1)),
        "outf": outf,
    }
    if "gamma" in p:
        g = np.asarray(p["gamma"], np.float32)
        be = np.asarray(p["beta"], np.float32)
        C = np.eye(L, dtype=np.float32) - 1.0 / L
        Cg = g[:, None] * C
        m["cgt"] = wf.add(Cg.T)
        m["w2"] = wf.add(1.0 / (L * g * g))
        m["bocg"] = wsc.add(Cg @ bo)
        m["beta"] = wsc.add(be)
    else:
        m["bo"] = wsc.add(bo[:, None])
    return m


def _host_prep(inputs):
    nf = np.asarray(inputs["nfeatures"], np.float32)
    ef = np.asarray(inputs["efeatures"], np.float32)
    src = np.asarray(inputs["src"], np.int64)
    dst = np.asarray(inputs["dst"], np.int64)

    perm = np.argsort(dst, kind="stable")
    dst_s = dst[perm]
    src_s = src[perm]
    ef_s = ef[perm]

    bounds = [0]
    for c in range(1, CORES):
        bounds.append(int(dst_s[c * NE // CORES]))
    bounds.append(NN)
    bounds = np.array(bounds, np.int64)
    assert np.all(np.diff(bounds) > 0)
    assert np.all(np.diff(bounds) <= NPC)

    inner = bounds[1:CORES]
    owner = np.searchsorted(inner, src_s, side="right")
    src_remap = NPC * owner + (src_s - bounds[owner])

    # pair boundaries: per core, per pair of 256 local nodes
    pair_lo = np.zeros((CORES, NPAIR), np.int64)
    pair_hi = np.zeros((CORES, NPAIR), np.int64)
    for c in range(CORES):
        for p in range(NPAIR):
            a = min(bounds[c] + 256 * p, bounds[c + 1])
            b = min(bounds[c] + 256 * (p + 1), bounds[c + 1])
            pair_lo[c, p] = np.searchsorted(dst_s, a, side="left")
            pair_hi[c, p] = np.searchsorted(dst_s, b, side="left")
    cnt = pair_hi - pair_lo
    pmax = int(np.ceil(cnt.max() / TE))
    T = NPAIR * pmax
    EP = T * TE

    cores = []
    for c in range(CORES):
        efp = np.zeros((EP, IN_EDGE), np.float32)
        d_rel = np.full((EP,), -1000.0, np.float32)
        srcR = np.zeros((EP,), np.int64)
        dstR = np.zeros((EP,), np.int64)
        for p in range(NPAIR):
            lo, hi = pair_lo[c, p], pair_hi[c, p]
            n = hi - lo
            s0 = p * pmax * TE
            efp[s0: s0 + n] = ef_s[lo:hi]
            d_rel[s0: s0 + n] = (dst_s[lo:hi] - bounds[c] - 256 * p).astype(np.float32)
            srcR[s0: s0 + n] = src_remap[lo:hi]
            dstR[s0: s0 + n] = NPC * c + dst_s[lo:hi] - bounds[c]

        def wrap(idx):
            a = idx.reshape(T, 32, 16).transpose(2, 0, 1).reshape(16, T * 32)
            return np.tile(a, (8, 1)).astype(np.int16)

        nfp = np.zeros((NPC, IN_NODE), np.float32)
        ncnt = bounds[c + 1] - bounds[c]
        nfp[:ncnt] = nf[bounds[c]: bounds[c + 1]]
        cores.append(dict(
            efT=np.ascontiguousarray(efp.T),
            nfT=np.ascontiguousarray(nfp.T),
            drel=np.ascontiguousarray(
                d_rel.reshape(T * 4, 128).T).astype(np.float32),
            srcW=wrap(srcR),
            dstW=wrap(dstR),
            ncnt=int(ncnt),
        ))

    # weights
    wf = _Packer(np.float32)
    wb = _Packer(ml_dtypes.bfloat16)
    wsc = _Packer(np.float32)
    W = {}
    W["enc_e"] = _prep_mlp(wf, wb, wsc, inputs["enc_edge_params"], [False])
    W["enc_n"] = _prep_mlp(wf, wb, wsc, inputs["enc_node_params"], [False])
    for i in range(ITERS):
        W[f"pe{i}"] = _prep_mlp(wf, wb, wsc, inputs["proc_edge_params"][i],
                                [False, True, True])
        W[f"pn{i}"] = _prep_mlp(wf, wb, wsc, inputs["proc_node_params"][i],
                                [False, False])
    W["out"] = _prep_mlp(wf, wb, wsc, inputs["out_params"], [False])
    W["ones"] = wf.add(np.ones((1, 128), np.float32))
    W["eps"] = wsc.add(np.full((1, 1), EPS, np.float32))
    W["ident"] = wsc.add(np.eye(128, dtype=np.float32))
    W["iota"] = wb.add(np.tile(np.arange(256, dtype=np.float32)[None, :], (128, 1)))

    meta = dict(T=T, pmax=pmax, W=W, bounds=bounds)
    return cores, wf.build(), wb.build(), wsc.build(), meta


# ---------------------------------------------------------------- device

def _build_program(T, pmax, W, xf, xb, xs):
    EP = T * TE
    nc = bacc.Bacc("TRN2", target_bir_lowering=False, debug=False,
                   num_devices=CORES, num_swdge_queues=2)

    p_efT = nc.declare_dram_parameter("efT", [IN_EDGE, EP], F32R, isOutput=False)
    p_nfT = nc.declare_dram_parameter("nfT", [IN_NODE, NPC], F32R, isOutput=False)
    p_drel = nc.declare_dram_parameter("drel", [128, T * 4], F32, isOutput=False)
    p_srcW = nc.declare_dram_parameter("srcW", [128, T * 32], I16, isOutput=False)
    p_dstW = nc.declare_dram_parameter("dstW", [128, T * 32], I16, isOutput=False)
    p_wf = nc.declare_dram_parameter("wf", [128, xf], F32R, isOutput=False)
    p_wb = nc.declare_dram_parameter("wb", [128, xb], BF16, isOutput=False)
    p_wsc = nc.declare_dram_parameter("wsc", [128, xs], F32, isOutput=False)
    p_out = nc.declare_dram_parameter("out", [OUT, NPC], F32, isOutput=True)

    rg = [list(range(CORES))]

    with tile.TileContext(nc) as tc:
        with tc.tile_pool(name="const", bufs=1) as cp, \
             tc.tile_pool(name="work", bufs=2) as wp, \
             tc.tile_pool(name="psA", bufs=3, space="PSUM") as psA, \
             tc.tile_pool(name="psV", bufs=1, space="PSUM") as psV, \
             tc.tile_pool(name="psT", bufs=2, space="PSUM") as psT, \
             tc.tile_pool(name="psP", bufs=2, space="PSUM") as psP, \
             tc.tile_pool(name="dram", bufs=1, space="DRAM") as dp:

            # resident tiles
            wf_t = cp.tile([128, xf], F32R)
            wb_t = cp.tile([128, xb], BF16)
            wsc_t = cp.tile([128, xs], F32)
            drel_t = cp.tile([128, T * 4], F32)
            srcW_t = cp.tile([128, T * 32], I16)
            dstW_t = cp.tile([128, T * 32], I16)
            nfT_t = cp.tile([IN_NODE, NPC], F32R)
            hT_t = cp.tile([128, NPC], F32R)       # h_own feature-major
            peT_t = cp.tile([128, NPC], F32R)      # pe_sum feature-major
            nc.sync.dma_start(wf_t[:], p_wf[:])
            nc.sync.dma_start(wb_t[:], p_wb[:])
            nc.sync.dma_start(wsc_t[:], p_wsc[:])
            nc.sync.dma_start(drel_t[:], p_drel[:])
            nc.sync.dma_start(srcW_t[:], p_srcW[:])
            nc.sync.dma_start(dstW_t[:], p_dstW[:])
            nc.sync.dma_start(nfT_t[:], p_nfT[:])

            e_dram = dp.tile([128, EP], F32R)
            h_bf_in = dp.tile([NPC, 128], BF16)
            h_bf_alls = [
                dp.tile([CORES * NPC, 128], BF16, addr_space="Shared",
                        name=f"h_bf_all{i}")
                for i in range(ITERS)]

            ident_ap = wsc_t[:, W["ident"]: W["ident"] + 128]
            iota_ap = wb_t[:, W["iota"]: W["iota"] + 256]
            ones_ap = wf_t[0:1, W["ones"]: W["ones"] + 128]

            def wf_ap(off, k, w):
                return wf_t[0:k, off: off + w]

            def wb_ap(off, k, w):
                return wb_t[0:k, off: off + w]

            def sc_ap(off, w=1, k=128):
                return wsc_t[0:k, off: off + w]

            def mlp_tile(m, terms, nfree=TE):
                """terms: list of (lhsT_ap [k,256], rhs_ap [k,nfree]).
                Returns a3 psum tile [outf, nfree] (pre-bias for no-LN;
                pre-Cg for LN)."""
                f1 = []
                for h in range(2):
                    a1 = psA.tile([128, nfree], F32, tag="a", bufs=3)
                    for j, (wap, rap) in enumerate(terms):
                        nc.tensor.matmul(
                            a1[:], wap[:, h * 128: (h + 1) * 128], rap,
                            start=(j == 0), stop=(j == len(terms) - 1))
                    f = wp.tile([128, nfree], F32R, tag="f1", bufs=4)
                    nc.scalar.activation(f[:], a1[:], LR,
                                         bias=sc_ap(m["b1"] + h), scale=1.0,
                                         alpha=ALPHA)
                    f1.append(f)
                f2 = []
                for h in range(2):
                    a2 = psA.tile([128, nfree], F32, tag="a", bufs=3)
                    for k in range(2):
                        nc.tensor.matmul(
                            a2[:],
                            wf_ap(m["wh"] + k * 256 + h * 128, 128, 128),
                            f1[k][:], start=(k == 0), stop=(k == 1))
                    f = wp.tile([128, nfree], F32R, tag="f2", bufs=4)
                    nc.scalar.activation(f[:], a2[:], LR,
                                         bias=sc_ap(m["b2"] + h), scale=1.0,
                                         alpha=ALPHA)
                    f2.append(f)
                outf = m["outf"]
                a3 = psA.tile([outf, nfree], F32, tag="a", bufs=3)
                for k in range(2):
                    nc.tensor.matmul(a3[:], wf_ap(m["wo"] + k * outf, 128, outf),
                                     f2[k][:], start=(k == 0), stop=(k == 1))
                return a3

            def ln_tail(m, a3, resid_ap, out_ap, nfree=TE):
                """LayerNorm + (optional residual). Writes to out_ap (f32r)."""
                s3 = wp.tile([128, nfree], F32R, tag="s3", bufs=2)
                nc.scalar.activation(s3[:], a3[:], IDENT, scale=1.0)
                xg = psA.tile([128, nfree], F32, tag="a", bufs=3)
                nc.tensor.matmul(xg[:], wf_ap(m["cgt"], 128, 128), s3[:],
                                 start=True, stop=True)
                xcg = wp.tile([128, nfree], F32R, tag="xcg", bufs=2)
                nc.vector.tensor_scalar(xcg[:], xg[:], sc_ap(m["bocg"]), None,
                                        op0=ADD)
                sq = wp.tile([128, nfree], F32R, tag="sq", bufs=2)
                nc.scalar.activation(sq[:], xcg[:], SQUARE, scale=1.0)
                vr = psV.tile([1, nfree], F32, tag="v", bufs=1)
                nc.tensor.matmul(vr[:], wf_ap(m["w2"], 128, 1), sq[:],
                                 start=True, stop=True)
                sd = wp.tile([1, nfree], F32, tag="sd", bufs=2)
                nc.scalar.activation(sd[:], vr[:], SQRT,
                                     bias=sc_ap(W["eps"], 1, 1), scale=1.0)
                rs = wp.tile([1, nfree], F32R, tag="rs", bufs=2)
                with nc.allow_low_precision(reason="rstd fp22 ok"):
                    nc.vector.reciprocal(rs[:], sd[:])
                rb = psA.tile([128, nfree], F32, tag="a", bufs=3)
                nc.tensor.matmul(rb[:], ones_ap, rs[:], start=True, stop=True)
                m1 = wp.tile([128, nfree], F32R, tag="m1", bufs=2)
                nc.vector.tensor_tensor(m1[:], xcg[:], rb[:], op=MULT)
                if resid_ap is None:
                    nc.vector.tensor_scalar(out_ap, m1[:], sc_ap(m["beta"]),
                                            None, op0=ADD)
                else:
                    nc.vector.scalar_tensor_tensor(out_ap, m1[:],
                                                   sc_ap(m["beta"]), resid_ap,
                                                   op0=ADD, op1=ADD)

            def hbf_update(nt):
                """h_ownT[:, nt*512:(nt+1)*512] -> bf16 row-major h_bf_in."""
                for s in range(4):
                    col = nt * TE + s * 128
                    trp = psT.tile([128, 128], F32, tag="tr", bufs=2)
                    nc.tensor.transpose(trp[:],
                                        hT_t[:, col: col + 128].bitcast(F32),
                                        ident_ap)
                    hb = wp.tile([128, 128], BF16, tag="hb", bufs=2)
                    nc.vector.tensor_copy(hb[:], trp[:])
                    nc.sync.dma_start(h_bf_in[col: col + 128, :], hb[:])

            # ---------------- encoders ----------------
            me = W["enc_e"]
            for t in range(T):
                eft = wp.tile([IN_EDGE, TE], F32R, tag="eft", bufs=3)
                nc.sync.dma_start(eft[:], p_efT[:, t * TE: (t + 1) * TE])
                (kind, off, kb), = me["blocks"]
                a3 = mlp_tile(me, [(wf_ap(off, kb, 256), eft[:])])
                enew = wp.tile([128, TE], F32R, tag="enew", bufs=3)
                ln_tail(me, a3, None, enew[:])
                nc.sync.dma_start(e_dram[:, t * TE: (t + 1) * TE], enew[:])

            mn = W["enc_n"]
            for nt in range(NPC // TE):
                cols = slice(nt * TE, (nt + 1) * TE)
                (kind, off, kb), = mn["blocks"]
                a3 = mlp_tile(mn, [(wf_ap(off, kb, 256), nfT_t[:, cols])])
                ln_tail(mn, a3, None, hT_t[:, cols])
                hbf_update(nt)

            # ---------------- processor iterations ----------------
            for it in range(ITERS):
                h_bf_all = h_bf_alls[it]
                nc.gpsimd.collective_compute(
                    "AllGather", mybir.AluOpType.bypass,
                    ins=[h_bf_in[:]], outs=[h_bf_all[:]], replica_groups=rg)

                mp = W[f"pe{it}"]
                (k0, o0, kb0), (k1, o1, kb1), (k2, o2, kb2) = mp["blocks"]
                pe_ps = None
                for t in range(T):
                    ecols = slice(t * TE, (t + 1) * TE)
                    e_old = wp.tile([128, TE], F32R, tag="eold", bufs=3)
                    nc.sync.dma_start(e_old[:], e_dram[:, ecols])
                    hs = wp.tile([128, 1, TE], BF16, tag="hs", bufs=3)
                    nc.gpsimd.dma_gather(
                        hs[:], h_bf_all[:], srcW_t[:, t * 32: (t + 1) * 32],
                        TE, TE, 128, elem_step=128, transpose=True)
                    hd = wp.tile([128, 1, TE], BF16, tag="hd", bufs=3)
                    nc.gpsimd.dma_gather(
                        hd[:], h_bf_all[:], dstW_t[:, t * 32: (t + 1) * 32],
                        TE, TE, 128, elem_step=128, transpose=True,
                        queue_num=1)
                    terms = [
                        (wf_ap(o0, kb0, 256), e_old[:]),
                        (wb_ap(o1, kb1, 256), hs[:, 0, :]),
                        (wb_ap(o2, kb2, 256), hd[:, 0, :]),
                    ]
                    a3 = mlp_tile(mp, terms)
                    enew = wp.tile([128, TE], F32R, tag="enew", bufs=3)
                    ln_tail(mp, a3, e_old[:], enew[:])
                    if it < ITERS - 1:
                        nc.sync.dma_start(e_dram[:, ecols], enew[:])
                    # segment-sum into per-pair psum accumulator
                    p = t // pmax
                    sp = t % pmax
                    if sp == 0:
                        pe_ps = psP.tile([128, 256], F32, tag="pe", bufs=2)
                    for s in range(4):
                        trp = psT.tile([128, 128], F32, tag="tr", bufs=2)
                        nc.tensor.transpose(
                            trp[:],
                            enew[:, s * 128: (s + 1) * 128].bitcast(F32),
                            ident_ap)
                        eem = wp.tile([128, 128], BF16, tag="eem", bufs=3)
                        nc.vector.tensor_copy(eem[:], trp[:])
                        msk = wp.tile([128, 256], BF16, tag="msk", bufs=3)
                        nc.gpsimd.tensor_scalar(
                            msk[:], iota_ap,
                            drel_t[:, t * 4 + s: t * 4 + s + 1], None, op0=EQ)
                        nc.tensor.matmul(pe_ps[:], eem[:], msk[:],
                                         start=(sp == 0 and s == 0),
                                         stop=(sp == pmax - 1 and s == 3))
                    if sp == pmax - 1:
                        nc.vector.tensor_copy(peT_t[:, 256 * p: 256 * (p + 1)],
                                              pe_ps[:])

                mq = W[f"pn{it}"]
                (ka, oa, kba), (kb_, ob, kbb) = mq["blocks"]
                for nt in range(NPC // TE):
                    cols = slice(nt * TE, (nt + 1) * TE)
                    terms = [
                        (wf_ap(oa, kba, 256), hT_t[:, cols]),
                        (wf_ap(ob, kbb, 256), peT_t[:, cols]),
                    ]
                    a3 = mlp_tile(mq, terms)
                    ln_tail(mq, a3, hT_t[:, cols], hT_t[:, cols])
                    if it < ITERS - 1:
                        hbf_update(nt)

            # ---------------- decoder ----------------
            mo = W["out"]
            (kind, off, kb), = mo["blocks"]
            for nt in range(NPC // TE):
                cols = slice(nt * TE, (nt + 1) * TE)
                a3 = mlp_tile(mo, [(wf_ap(off, kb, 256), hT_t[:, cols])])
                o = wp.tile([OUT, TE], F32, tag="o", bufs=2)
                nc.vector.tensor_scalar(o[:], a3[:], sc_ap(mo["bo"], 1, OUT),
                                        None, op0=ADD)
                nc.sync.dma_start(p_out[:, cols], o[:])

    nc.compile()
    return nc


_CACHE = {}


def kernel(**inputs):
    cores, wf_m, wb_m, wsc_m, meta = _host_prep(inputs)
    T, pmax, W = meta["T"], meta["pmax"], meta["W"]
    key = (T, pmax, wf_m.shape[1], wb_m.shape[1], wsc_m.shape[1])
    if key not in _CACHE:
        _CACHE[key] = _build_program(T, pmax, W, wf_m.shape[1], wb_m.shape[1],
                                     wsc_m.shape[1])
    nc = _CACHE[key]

    in_maps = []
    for c in range(CORES):
        d = cores[c]
        in_maps.append(dict(
            efT=d["efT"], nfT=d["nfT"], drel=d["drel"],
            srcW=d["srcW"], dstW=d["dstW"],
            wf=wf_m, wb=wb_m, wsc=wsc_m,
        ))
    res = run_bass_kernel_spmd(nc, in_maps, core_ids=list(range(CORES)))
    out = np.concatenate(
        [res.results[c]["out"][:, : cores[c]["ncnt"]].T for c in range(CORES)],
        axis=0)
    return out.astype(np.float32)
